# revision 1
# baseline (speedup 1.0000x reference)
"""JacobiKAN layer on 8 TRN2 NeuronCores — data-parallel Bass/Tile kernel.

  reference: out = silu(LN(silu(x) @ W.T + einsum('bid,iod->bo', jacobi(tanh x), C)))
  x [8192, 1024], W [1024, 1024], C [1024, 1024, 9]; order-8 Jacobi (a=b=1).

Strategy
  - Shard the token dim B=8192 across 8 cores (1024 rows each); weights
    replicated.  No collectives.
  - Reformulate the Jacobi einsum in the monomial basis:
        y = sum_m t^m @ D_m,   D_m = sum_d M[d,m] C[:,:,d]
    (M = Jacobi->monomial change of basis, host-precomputed).  The m=0 term
    is x-independent: a bias row v = sum_i D_0[i,:], injected with a K=1
    ones-matmul.  Device computes t^2..t^8 with 3 ScalarE squares + 4
    VectorE mults per tile — half the elementwise cost of the 2-op-per-
    degree Jacobi recurrence.
  - All matmuls in fp32r (fp32 with 12-bit mantissa, 1 PE cycle/row at
    N=512 — bf16 speed).  End-to-end scale-relative error vs the fp32
    reference is ~2e-4 (basis term dominates z, so the monomial
    amplification stays subdominant).
  - PSUM holds half the per-core output: two o-half passes, 8 banks of
    [128b, 512o] each; z parked in SBUF between passes; LayerNorm+SiLU
    fused at the end (bn_stats/bn_aggr + one ScalarE Silu with per-row
    scale/bias).
"""
import os
import sys
from contextlib import ExitStack

import numpy as np

for _p in ("/opt/trn_rl_repo",):
    if _p not in sys.path and os.path.isdir(_p):
        sys.path.append(_p)

import concourse.bacc as bacc
import concourse.mybir as mybir
import concourse.tile as tile
from concourse.bass_utils import run_bass_kernel_spmd
from neuronxcc.starfish.support.dtype import (
    static_cast_fp32_to_fp32r,
    static_cast_fp32r_to_fp32,
)

F32 = mybir.dt.float32
F32R = mybir.dt.float32r
AF = mybir.ActivationFunctionType
ALU = mybir.AluOpType

N_CORES = 8
B_FULL, IN_F, OUT_F, ORDER = 8192, 1024, 1024, 8
B_CORE = B_FULL // N_CORES          # 1024 rows per core
LN_EPS = 1e-5
N_K = IN_F // 128                   # 8 contraction chunks
N_J = B_CORE // 128                 # 8 output row-tiles per core
N_OH = 2                            # two 512-wide o halves (PSUM capacity)


def _q(a):
    """Round to fp32r (12-bit mantissa) — what the PE array consumes."""
    return static_cast_fp32r_to_fp32(
        static_cast_fp32_to_fp32r(np.ascontiguousarray(a, np.float32)))


def _monomial_matrix():
    """M[d, m]: P_d^{(1,1)}(t) = sum_m M[d,m] t^m (reference recurrence)."""
    a = b = 1.0
    M = np.zeros((ORDER + 1, ORDER + 1))
    M[0, 0] = 1.0
    M[1, 1] = (a + b + 2.0) / 2.0
    M[1, 0] = (a - b) / 2.0
    for i in range(2, ORDER + 1):
        th_k = (2 * i + a + b) * (2 * i + a + b - 1) / (2 * i * (i + a + b))
        th_k1 = ((2 * i + a + b - 1) * (a * a - b * b)
                 / (2 * i * (i + a + b) * (2 * i + a + b - 2)))
        th_k2 = ((i + a - 1) * (i + b - 1) * (2 * i + a + b)
                 / (i * (i + a + b) * (2 * i + a + b - 2)))
        M[i, 1:] += th_k * M[i - 1, :-1]
        M[i, :] += th_k1 * M[i - 1, :]
        M[i, :] -= th_k2 * M[i - 2, :]
    return M


def _build_program(general_ln, reps=1):
    """reps>1 wraps the whole body in a device-side For_i so wall-clock
    timing can amortize the PJRT dispatch overhead (test-only)."""
    import contextlib
    nc = bacc.Bacc("TRN2", target_bir_lowering=False, debug=False)

    xt_d = nc.dram_tensor("xt", [IN_F, B_CORE], F32, kind="ExternalInput").ap()
    dm_d = nc.dram_tensor("dmono", [N_OH, N_K, ORDER, 128, 512], F32R,
                          kind="ExternalInput").ap()
    wt_d = nc.dram_tensor("wtp", [N_OH, N_K, 128, 512], F32R,
                          kind="ExternalInput").ap()
    v_d = nc.dram_tensor("vrow", [1, OUT_F], F32R, kind="ExternalInput").ap()
    one_d = nc.dram_tensor("onerow", [1, 128], F32R, kind="ExternalInput").ap()
    if general_ln:
        lnw_d = nc.dram_tensor("lnw", [1, OUT_F], F32, kind="ExternalInput").ap()
        lnb_d = nc.dram_tensor("lnb", [1, OUT_F], F32, kind="ExternalInput").ap()
    out_d = nc.dram_tensor("out", [B_CORE, OUT_F], F32,
                           kind="ExternalOutput").ap()

    with tile.TileContext(nc) as tc:
        with ExitStack() as ctx:
            const = ctx.enter_context(tc.tile_pool(name="const", bufs=1))
            xload = ctx.enter_context(tc.tile_pool(name="xload", bufs=2))
            acts = ctx.enter_context(tc.tile_pool(name="acts", bufs=1))
            zpark = ctx.enter_context(tc.tile_pool(name="zpark", bufs=1))
            pwp = ctx.enter_context(tc.tile_pool(name="pwp", bufs=2))
            dstr = ctx.enter_context(tc.tile_pool(name="dstr", bufs=2))
            outp = ctx.enter_context(tc.tile_pool(name="outp", bufs=2))
            stat = ctx.enter_context(tc.tile_pool(name="stat", bufs=4))
            psum = ctx.enter_context(tc.tile_pool(name="psum", bufs=1,
                                                  space="PSUM"))

            ones_t = const.tile([1, 128], F32R)
            nc.sync.dma_start(ones_t, one_d)
            eps_t = const.tile([128, 1], F32)
            nc.vector.memset(eps_t, LN_EPS)
            v_t = const.tile([1, OUT_F], F32R)
            nc.sync.dma_start(v_t, v_d)
            if general_ln:
                import concourse.bass as bass
                lnw_t = const.tile([128, OUT_F], F32)
                nc.sync.dma_start(lnw_t, bass.AP(
                    tensor=lnw_d.tensor, offset=lnw_d.offset,
                    ap=[[0, 128]] + list(lnw_d.ap[1:])))
                lnb_t = const.tile([128, OUT_F], F32)
                nc.sync.dma_start(lnb_t, bass.AP(
                    tensor=lnb_d.tensor, offset=lnb_d.offset,
                    ap=[[0, 128]] + list(lnb_d.ap[1:])))

            loop_cm = (tc.For_i(0, reps, 1) if reps > 1
                       else contextlib.nullcontext())
            with loop_cm:
                _emit_body(nc, tc, xload, acts, zpark, pwp, dstr, outp, stat,
                           psum, xt_d, dm_d, wt_d, out_d, ones_t, v_t, eps_t,
                           lnw_t if general_ln else None,
                           lnb_t if general_ln else None)

    nc.compile()
    return nc


def _emit_body(nc, tc, xload, acts, zpark, pwp, dstr, outp, stat, psum,
               xt_d, dm_d, wt_d, out_d, ones_t, v_t, eps_t, lnw_t, lnb_t):
    general_ln = lnw_t is not None
    if True:
        if True:
            # T[k] = tanh(x.T chunk), SIL[k] = silu(x.T chunk); fp32r outs
            # (matmul lhsT producers must be fp32r-typed for the verifier).
            T = []
            SIL = []
            for k in range(N_K):
                xt_t = xload.tile([128, B_CORE], F32, name=f"xt_{k}",
                                  tag="xt")
                nc.sync.dma_start(xt_t, xt_d[128 * k:128 * (k + 1), :])
                t_t = acts.tile([128, B_CORE], F32R, name=f"tanh_{k}",
                                tag=f"tanh_{k}")
                nc.scalar.activation(t_t, xt_t, AF.Tanh)
                s_t = acts.tile([128, B_CORE], F32R, name=f"sil_{k}",
                                tag=f"sil_{k}")
                nc.scalar.activation(s_t, xt_t, AF.Silu)
                T.append(t_t)
                SIL.append(s_t)

            z = [zpark.tile([128, OUT_F], F32, name=f"z_{j}", tag=f"z_{j}")
                 for j in range(N_J)]

            for oh in range(N_OH):
                osl = slice(512 * oh, 512 * (oh + 1))
                ps = [psum.tile([128, 512], F32, name=f"ps_{oh}_{j}",
                                tag=f"ps_{j}") for j in range(N_J)]
                for k in range(N_K):
                    dm_t = dstr.tile([128, ORDER, 512], F32R,
                                     name=f"dm_{oh}_{k}", tag="dm")
                    # dmono[oh, k] is [ORDER, 128, 512]; partition dim is
                    # axis 1 of the slice -> per-partition rows of 512.
                    src = dm_d[oh, k].rearrange("m p o -> p m o")
                    nc.sync.dma_start(dm_t, src)
                    wt_t = dstr.tile([128, 512], F32R,
                                     name=f"wt_{oh}_{k}", tag="wt")
                    nc.sync.dma_start(wt_t, wt_d[oh, k])

                    for bh in range(2):
                        bsl = slice(512 * bh, 512 * (bh + 1))
                        tk = T[k][:, bsl]           # f32r [128, 512]
                        tkf = tk.bitcast(F32)
                        pw = pwp.tile([128, 7, 512], F32R,
                                      name=f"pw_{oh}_{k}_{bh}", tag="pw")
                        # slots: 0:t2 1:t3 2:t4 3:t5 4:t6 5:t7 6:t8
                        nc.scalar.activation(pw[:, 0, :], tkf, AF.Square)
                        nc.scalar.activation(pw[:, 2, :],
                                             pw[:, 0, :].bitcast(F32),
                                             AF.Square)
                        nc.scalar.activation(pw[:, 6, :],
                                             pw[:, 2, :].bitcast(F32),
                                             AF.Square)
                        nc.vector.tensor_mul(pw[:, 1, :],
                                             pw[:, 0, :].bitcast(F32), tkf)
                        nc.vector.tensor_mul(pw[:, 3, :],
                                             pw[:, 2, :].bitcast(F32), tkf)
                        nc.vector.tensor_mul(pw[:, 4, :],
                                             pw[:, 2, :].bitcast(F32),
                                             pw[:, 0, :].bitcast(F32))
                        nc.vector.tensor_mul(pw[:, 5, :],
                                             pw[:, 2, :].bitcast(F32),
                                             pw[:, 1, :].bitcast(F32))

                        for j4 in range(4):
                            j = 4 * bh + j4
                            jsl = slice(128 * j, 128 * (j + 1))
                            j4sl = slice(128 * j4, 128 * (j4 + 1))
                            if k == 0:
                                # K=1 ones-matmul injects the m=0 bias row;
                                # writes every element -> starts the group.
                                nc.tensor.matmul(ps[j], ones_t, v_t[:, osl],
                                                 start=True, stop=False)
                            nc.tensor.matmul(ps[j], SIL[k][:, jsl], wt_t,
                                             start=False, stop=False)
                            nc.tensor.matmul(ps[j], T[k][:, jsl],
                                             dm_t[:, 0, :],
                                             start=False, stop=False)
                            for m in range(2, ORDER + 1):
                                last = (k == N_K - 1 and m == ORDER)
                                nc.tensor.matmul(
                                    ps[j],
                                    pw[:, m - 2, j4sl],
                                    dm_t[:, m - 1, :],
                                    start=False, stop=last)
                # park this o-half
                for j in range(N_J):
                    nc.scalar.copy(z[j][:, osl], ps[j])

            # LayerNorm over o (=free dim) + silu, per row-tile.
            for j in range(N_J):
                st = stat.tile([128, 2, 6], F32, name=f"st_{j}", tag="st")
                nc.vector.bn_stats(st[:, 0, :], z[j][:, 0:512])
                nc.vector.bn_stats(st[:, 1, :], z[j][:, 512:1024])
                mv = stat.tile([128, 2], F32, name=f"mv_{j}", tag="mv")
                nc.vector.bn_aggr(mv, st)
                sd = stat.tile([128, 1], F32, name=f"sd_{j}", tag="sd")
                nc.scalar.activation(sd, mv[:, 1:2], AF.Sqrt, bias=eps_t)
                r = stat.tile([128, 1], F32, name=f"r_{j}", tag="r")
                nc.vector.reciprocal(r, sd)
                nb = stat.tile([128, 1], F32, name=f"nb_{j}", tag="nb")
                nc.vector.scalar_tensor_tensor(nb, mv[:, 0:1], -1.0, r,
                                               op0=ALU.mult, op1=ALU.mult)
                o_t = outp.tile([128, OUT_F], F32, name=f"o_{j}", tag="o")
                if general_ln:
                    zn = outp.tile([128, OUT_F], F32, name=f"zn_{j}",
                                   tag="zn")
                    nc.scalar.activation(zn, z[j], AF.Identity,
                                         bias=nb, scale=r)
                    nc.vector.tensor_mul(zn, zn, lnw_t)
                    nc.vector.tensor_add(zn, zn, lnb_t)
                    nc.scalar.activation(o_t, zn, AF.Silu)
                else:
                    nc.scalar.activation(o_t, z[j], AF.Silu,
                                         bias=nb, scale=r)
                nc.sync.dma_start(out_d[128 * j:128 * (j + 1), :], o_t)


_PROG_CACHE = {}


def _get_program(general_ln):
    if general_ln not in _PROG_CACHE:
        _PROG_CACHE[general_ln] = _build_program(general_ln)
    return _PROG_CACHE[general_ln]


def _prep_shared(base_weights, jacobi_coeff, ln_weight, ln_bias, general_ln):
    M = _monomial_matrix()
    # D[:, :, m] = sum_d M[d, m] * C[:, :, d]
    D = np.einsum("dm,iod->iom", M, jacobi_coeff.astype(np.float64))

    v = D[:, :, 0].sum(axis=0).astype(np.float32).reshape(1, OUT_F)

    # dmono[oh, k, m-1, p, o] = D[128k+p, 512oh+o, m]
    Dp = np.transpose(D[:, :, 1:].astype(np.float32), (2, 0, 1))  # [8, in, out]
    Dp = Dp.reshape(ORDER, N_K, 128, N_OH, 512)
    dmono = np.ascontiguousarray(np.transpose(Dp, (3, 1, 0, 2, 4)))
    dmono = _q(dmono)

    # wtp[oh, k, p, o] = W[512oh+o, 128k+p]
    Wt = np.ascontiguousarray(base_weights.T)                # [in, out]
    Wt = Wt.reshape(N_K, 128, N_OH, 512)
    wtp = np.ascontiguousarray(np.transpose(Wt, (2, 0, 1, 3)))
    wtp = _q(wtp)

    shared = {
        "dmono": dmono,
        "wtp": wtp,
        "vrow": _q(v),
        "onerow": np.ones((1, 128), np.float32),
    }
    if general_ln:
        shared["lnw"] = np.ascontiguousarray(
            ln_weight.reshape(1, OUT_F).astype(np.float32))
        shared["lnb"] = np.ascontiguousarray(
            ln_bias.reshape(1, OUT_F).astype(np.float32))
    return shared


def kernel(x, base_weights, jacobi_coeff, ln_weight, ln_bias):
    x = np.asarray(x, np.float32).reshape(B_FULL, IN_F)
    base_weights = np.asarray(base_weights, np.float32)
    jacobi_coeff = np.asarray(jacobi_coeff, np.float32)
    ln_weight = np.asarray(ln_weight, np.float32)
    ln_bias = np.asarray(ln_bias, np.float32)

    general_ln = not (np.all(ln_weight == 1.0) and np.all(ln_bias == 0.0))

    nc = _get_program(general_ln)
    shared = _prep_shared(base_weights, jacobi_coeff, ln_weight, ln_bias,
                          general_ln)

    in_maps = []
    for c in range(N_CORES):
        xt = np.ascontiguousarray(
            x[B_CORE * c:B_CORE * (c + 1), :].T)     # [in, b_core]
        in_maps.append({"xt": xt, **shared})

    res = run_bass_kernel_spmd(nc, in_maps, core_ids=list(range(N_CORES)))
    out = np.concatenate([res.results[c]["out"] for c in range(N_CORES)],
                         axis=0)
    return out.astype(np.float32)


if __name__ == "__main__":
    rng = np.random.default_rng(1)
    demo = {
        "x": rng.standard_normal((B_FULL, IN_F)).astype(np.float32),
        "base_weights": rng.standard_normal((OUT_F, IN_F)).astype(np.float32) * 0.04,
        "jacobi_coeff": (rng.standard_normal((IN_F, OUT_F, ORDER + 1))
                         / (IN_F * (ORDER + 1))).astype(np.float32),
        "ln_weight": np.ones(OUT_F, np.float32),
        "ln_bias": np.zeros(OUT_F, np.float32),
    }
    o = kernel(**demo)
    print("kernel output:", o.shape, o.dtype, float(np.abs(o).mean()))



# revision 36
# speedup vs baseline: 1.7398x; 1.7398x over previous
"""JacobiKAN layer on 8 TRN2 NeuronCores — data-parallel Bass/Tile kernel.

  reference: out = silu(LN(silu(x) @ W.T + einsum('bid,iod->bo', jacobi(tanh x), C)))
  x [8192, 1024], W [1024, 1024], C [1024, 1024, 9]; order-8 Jacobi (a=b=1).

Strategy
  - Shard the token dim B=8192 across 8 cores (1024 rows each); weights
    replicated.  No collectives.
  - Rewrite the Jacobi einsum in an 8-function polynomial feature basis
    g_1..g_8 of degrees 1..8 in t=tanh(x).  The basis is built on-device
    by a chain of ScalarE Squares (with bias shifts) and VectorE
    scalar_tensor_tensor ops whose shift/correction scalars are solved
    host-side so that each g_m equals the EXACT monic orthogonal
    polynomial q_m of the tanh-Gaussian measure (modulo lower-span
    components, which are harmless).  Orthogonality keeps the projected
    coefficients small, so fp8 quantization noise stays ~2% of the
    (tiny) Jacobi term instead of being amplified by the ill-conditioned
    monomial representation.
  - Features and coefficients are fp8 e4m3; the 8 coefficient matmuls
    per tile collapse to 4 DoubleRow matmuls (2 fp8 contractions/cell).
    The silu(x) @ W.T term runs in bf16.  Coefficients are quantized
    high-order-first with error feedback (residuals projected onto the
    lower basis), and the m=0 constants fold into a bias row injected by
    a K=1 ones-matmul.
  - Everything is scaled by 2^10 (W, coeffs, bias row) so the fp8/bf16
    ranges are centered; LayerNorm is scale-invariant (eps scaled by
    2^20), so no descale op exists anywhere.
  - LayerNorm: bn_stats/bn_aggr per row-tile, then a batched Newton
    rsqrt on VectorE (bit-hack seed + 2 iterations) — no ScalarE Sqrt,
    so the whole kernel uses one activation table set (silu_and_others):
    zero table switches.
  - PSUM holds half the per-core output: per o-half, 8 banks of
    [128b, 512o]; z parked to SBUF in f32; final Silu fused with the
    per-row scale/bias on ScalarE; f32 output.
"""
import os
import sys
from contextlib import ExitStack

import numpy as np

for _p in ("/opt/trn_rl_repo",):
    if _p not in sys.path and os.path.isdir(_p):
        sys.path.append(_p)

import ml_dtypes

import concourse.bacc as bacc
import concourse.mybir as mybir
import concourse.tile as tile
from concourse.bass_utils import run_bass_kernel_spmd

F32 = mybir.dt.float32
BF16 = mybir.dt.bfloat16
F8 = mybir.dt.float8e4
U32 = mybir.dt.uint32
AF = mybir.ActivationFunctionType
ALU = mybir.AluOpType
DR = mybir.MatmulPerfMode.DoubleRow

N_CORES = 8
B_FULL, IN_F, OUT_F, ORDER = 8192, 1024, 1024, 8
B_CORE = B_FULL // N_CORES          # 1024 rows per core
LN_EPS = 1e-5
N_K = IN_F // 128                   # 8 contraction chunks
N_J = B_CORE // 128                 # 8 output row-tiles per core
N_OH = 2                            # two 512-wide o halves (PSUM capacity)
N_PAIR = 4                          # DoubleRow feature pairs
SC = 2.0 ** 10                      # global output scale (LN absorbs it)
SIG8 = 16.0                         # extra storage scale for the q8 feature
MAGIC = float(np.frombuffer(np.uint32(0x5F3759DF).tobytes(),
                            np.float32)[0])


# --------------------------------------------------------------------------
# Host-side basis construction
# --------------------------------------------------------------------------

def _monomial_matrix():
    """M[d, m]: P_d^{(1,1)}(t) = sum_m M[d,m] t^m (reference recurrence)."""
    a = b = 1.0
    M = np.zeros((ORDER + 1, ORDER + 1))
    M[0, 0] = 1.0
    M[1, 1] = (a + b + 2.0) / 2.0
    M[1, 0] = (a - b) / 2.0
    for i in range(2, ORDER + 1):
        th_k = (2 * i + a + b) * (2 * i + a + b - 1) / (2 * i * (i + a + b))
        th_k1 = ((2 * i + a + b - 1) * (a * a - b * b)
                 / (2 * i * (i + a + b) * (2 * i + a + b - 2)))
        th_k2 = ((i + a - 1) * (i + b - 1) * (2 * i + a + b)
                 / (i * (i + a + b) * (2 * i + a + b - 2)))
        M[i, 1:] += th_k * M[i - 1, :-1]
        M[i, :] += th_k1 * M[i - 1, :]
        M[i, :] -= th_k2 * M[i - 2, :]
    return M


def _tanh_moments(nmax):
    xs = np.linspace(-12.0, 12.0, 2_000_001)
    w = np.exp(-xs * xs / 2.0)
    w /= w.sum()
    t = np.tanh(xs)
    return np.array([(w * t ** k).sum() for k in range(nmax + 1)])


def _build_basis():
    """Solve the chain shift/correction scalars so each device feature is
    the exact monic orthogonal polynomial (mod lower-span parts).

    Returns (params dict, stored-basis poly coeff rows B [9, 9]).
    """
    mom = _tanh_moments(2 * ORDER + 2)

    def ip(p, q):
        r = np.convolve(p, q)
        return float(sum(c * mom[i] for i, c in enumerate(r)))

    # monic orthogonal polys under the tanh-Gaussian measure
    q = [np.zeros(ORDER + 1) for _ in range(ORDER + 1)]
    q[0][0] = 1.0
    for m in range(1, ORDER + 1):
        v = np.zeros(ORDER + 1)
        v[m] = 1.0
        for l in range(m):
            v -= (ip(v, q[l]) / ip(q[l], q[l])) * q[l]
        q[m] = v

    def pm(p, deg):                  # poly coefficient at degree `deg`
        return p[deg] if deg < len(p) else 0.0

    t_ = np.zeros(2); t_[1] = 1.0
    g2 = np.zeros(3); g2[2] = 1.0

    a = q[3][1]                                  # g3 = (g2 + a) t == q3
    g3 = np.convolve(np.r_[a, 0.0, 1.0], t_)
    b = q[4][2] / 2.0                            # g4 = (g2 + b)^2
    g4 = np.convolve(np.r_[b, 0.0, 1.0], np.r_[b, 0.0, 1.0])
    # Products u5/u6/u7 are plain Pool tensor_tensor muls (no shift --
    # the missing shift is a lower-span residue, absorbed by projection).
    # g5 = (g3 * s5) + g4 * t  == q5 + span{t}
    s5 = q[5][3] - pm(g4, 2)
    u5 = np.convolve(g4, t_)
    g5 = s5 * np.pad(g3, (0, len(u5) - len(g3))) + u5
    # g6 = (g4 * s6) + g4 * g2  == q6 + span{t^2, 1}
    s6 = q[6][4] - pm(g4, 2)
    u6 = np.convolve(g4, g2)
    g6 = s6 * np.pad(g4, (0, len(u6) - len(g4))) + u6
    # g7 = (g5 * s7) + g4 * g3  == q7 + span{t^3, t}
    u7 = np.convolve(g4, g3)
    s7 = q[7][5] - pm(u7, 5)
    g7 = s7 * np.pad(g5, (0, len(u7) - len(g5))) + u7
    # g8 = (g6 * s8) + (g4 + h)^2  == q8 + span{t^2, 1}
    A = np.convolve(g4, g4)
    s8 = q[8][6] - pm(A, 6)
    h = (q[8][4] - pm(A, 4) - s8 * pm(g6, 4)) / 2.0
    u8q = g4 + np.r_[h, np.zeros(4)]
    u8 = np.convolve(u8q, u8q)
    g8 = SIG8 * (s8 * np.pad(g6, (0, len(u8) - len(g6))) + u8)

    basis = [np.r_[1.0], t_, g2, g3, g4, g5, g6, g7, g8]
    B = np.zeros((ORDER + 1, ORDER + 1))
    for i, p in enumerate(basis):
        B[i, :len(p)] = p
    G = np.array([[ip(basis[i], basis[j]) for j in range(ORDER + 1)]
                  for i in range(ORDER + 1)])
    mono = [np.eye(ORDER + 1)[k] for k in range(ORDER + 1)]
    Gx = np.array([[ip(basis[i], mono[j]) for j in range(ORDER + 1)]
                   for i in range(ORDER + 1)])
    params = dict(a=a, b=b, s5=s5, s6=s6, s7=s7, h=h, s8=s8)
    return params, B, G, Gx


_BASIS_CACHE = None


def _basis():
    global _BASIS_CACHE
    if _BASIS_CACHE is None:
        _BASIS_CACHE = _build_basis()
    return _BASIS_CACHE


def _f8r(x):
    return np.asarray(x, ml_dtypes.float8_e4m3).astype(np.float64)


def _prep_shared(base_weights, jacobi_coeff, ln_weight, ln_bias, general_ln):
    params, B, G, Gx = _basis()
    M = _monomial_matrix()
    # D[:, :, m] = monomial coeffs; project onto stored basis
    D = np.einsum("dm,iod->iom", M, jacobi_coeff.astype(np.float64))
    P = np.linalg.solve(G, Gx)                 # mono -> basis coeffs
    Dp = np.einsum("pm,iom->iop", P, D)        # [in, out, 9]

    # error-feedback quantization, high order -> low
    cur = Dp.copy()
    Dq = np.zeros((IN_F, OUT_F, ORDER), dtype=np.float64)
    for m in range(ORDER, 0, -1):
        qz = _f8r(SC * cur[:, :, m]) / SC
        Dq[:, :, m - 1] = qz
        r = cur[:, :, m] - qz
        sol = np.linalg.solve(G[:m, :m], G[:m, m])
        cur[:, :, :m] += r[:, :, None] * sol[None, None, :]
    v = (SC * cur[:, :, 0].sum(axis=0)).astype(np.float32)   # bias row

    # dm[k, p, oh, pair, slot, o] = SC*Dq[128k+p, 512oh+o, 2pair+slot]
    Ds = (SC * Dq).astype(ml_dtypes.float8_e4m3)
    Ds = Ds.reshape(N_K, 128, N_OH, 512, N_PAIR, 2)
    dm = np.ascontiguousarray(np.transpose(Ds, (0, 1, 2, 4, 5, 3)))

    # wtp[k, p, oh, o] = SC*W[512oh+o, 128k+p]  (bf16)
    Wt = np.ascontiguousarray(SC * base_weights.T.astype(np.float64))
    wtp = Wt.reshape(N_K, 128, N_OH, 512).astype(ml_dtypes.bfloat16)

    shared = {
        "dmono": dm,
        "wtp": wtp,
        "vrow": v.reshape(1, OUT_F).astype(ml_dtypes.bfloat16),
        "onerow": np.ones((1, 128), ml_dtypes.bfloat16),
    }
    if general_ln:
        shared["lnw"] = np.ascontiguousarray(
            ln_weight.reshape(1, OUT_F).astype(np.float32))
        shared["lnb"] = np.ascontiguousarray(
            ln_bias.reshape(1, OUT_F).astype(np.float32))
    return shared


# --------------------------------------------------------------------------
# Device program
# --------------------------------------------------------------------------

def _build_program(general_ln, reps=1, debug_taps=False):
    """reps>1 wraps the whole body in a device-side For_i so wall-clock
    timing can amortize the PJRT dispatch overhead (test-only)."""
    import contextlib
    nc = bacc.Bacc("TRN2", target_bir_lowering=False, debug=False)

    xt_d = nc.dram_tensor("xt", [IN_F, B_CORE], BF16, kind="ExternalInput").ap()
    dm_d = nc.dram_tensor("dmono", [N_K, 128, N_OH, N_PAIR, 2, 512], F8,
                          kind="ExternalInput").ap()
    wt_d = nc.dram_tensor("wtp", [N_K, 128, N_OH, 512], BF16,
                          kind="ExternalInput").ap()
    v_d = nc.dram_tensor("vrow", [1, OUT_F], BF16, kind="ExternalInput").ap()
    one_d = nc.dram_tensor("onerow", [1, 128], BF16, kind="ExternalInput").ap()
    if general_ln:
        lnw_d = nc.dram_tensor("lnw", [1, OUT_F], F32, kind="ExternalInput").ap()
        lnb_d = nc.dram_tensor("lnb", [1, OUT_F], F32, kind="ExternalInput").ap()
    out_d = nc.dram_tensor("out", [B_CORE, OUT_F], F32,
                           kind="ExternalOutput").ap()
    dbg = None
    if debug_taps:
        dbg = {
            "pw0": nc.dram_tensor("dbg_pw0", [128, ORDER, B_CORE], F8,
                                  kind="ExternalOutput").ap(),
            "z0": nc.dram_tensor("dbg_z0", [128, OUT_F], F32,
                                 kind="ExternalOutput").ap(),
            "sm": nc.dram_tensor("dbg_sm", [128, N_J, 2], F32,
                                 kind="ExternalOutput").ap(),
            "s2": nc.dram_tensor("dbg_s2", [128, N_J, 2], F32,
                                 kind="ExternalOutput").ap(),
            "r": nc.dram_tensor("dbg_r", [128, N_J], F32,
                                kind="ExternalOutput").ap(),
            "nb": nc.dram_tensor("dbg_nb", [128, N_J], F32,
                                 kind="ExternalOutput").ap(),
        }

    with tile.TileContext(nc) as tc:
        with ExitStack() as ctx:
            const = ctx.enter_context(tc.tile_pool(name="const", bufs=1))
            xload = ctx.enter_context(tc.tile_pool(name="xload", bufs=2))
            feats = ctx.enter_context(tc.tile_pool(name="feats", bufs=1))
            scr = ctx.enter_context(tc.tile_pool(name="scr", bufs=2))
            zpark = ctx.enter_context(tc.tile_pool(name="zpark", bufs=1))
            dstr = ctx.enter_context(tc.tile_pool(name="dstr", bufs=3))
            outp = ctx.enter_context(tc.tile_pool(name="outp", bufs=4))
            stat = ctx.enter_context(tc.tile_pool(name="stat", bufs=2))
            psum = ctx.enter_context(tc.tile_pool(name="psum", bufs=1,
                                                  space="PSUM"))

            ones_t = const.tile([1, 128], BF16)
            nc.sync.dma_start(ones_t, one_d)
            v_t = const.tile([1, OUT_F], BF16)
            nc.sync.dma_start(v_t, v_d)
            magic_t = const.tile([128, N_J], F32)
            nc.vector.memset(magic_t, MAGIC)
            pbasis = _basis()[0]
            bias_b = const.tile([128, 1], F32)
            nc.vector.memset(bias_b, pbasis["b"])
            bias_h = const.tile([128, 1], F32)
            nc.vector.memset(bias_h, 4.0 * pbasis["h"])
            zero_t = const.tile([128, 512], F32)
            nc.vector.memset(zero_t, 0.0)
            if general_ln:
                import concourse.bass as bass
                lnw_t = const.tile([128, OUT_F], F32)
                nc.sync.dma_start(lnw_t, bass.AP(
                    tensor=lnw_d.tensor, offset=lnw_d.offset,
                    ap=[[0, 128]] + list(lnw_d.ap[1:])))
                lnb_t = const.tile([128, OUT_F], F32)
                nc.sync.dma_start(lnb_t, bass.AP(
                    tensor=lnb_d.tensor, offset=lnb_d.offset,
                    ap=[[0, 128]] + list(lnb_d.ap[1:])))

            loop_cm = (tc.For_i(0, reps, 1) if reps > 1
                       else contextlib.nullcontext())
            with loop_cm:
                _emit_body(nc, tc, xload, feats, scr, zpark, dstr, outp, stat,
                           psum, xt_d, dm_d, wt_d, out_d, ones_t, v_t,
                           magic_t, bias_b, bias_h, zero_t,
                           lnw_t if general_ln else None,
                           lnb_t if general_ln else None, dbg=dbg)

    nc.compile()
    return nc


def _emit_body(nc, tc, xload, feats, scr, zpark, dstr, outp, stat, psum,
               xt_d, dm_d, wt_d, out_d, ones_t, v_t, magic_t, bias_b, bias_h,
               zero_t, lnw_t, lnb_t, dbg=None):
    general_ln = lnw_t is not None
    p, _, _, _ = _basis()
    a, b, s5, s6, s7, h, s8 = (p["a"], p["b"], p["s5"], p["s6"], p["s7"],
                               p["h"], p["s8"])

    # ---- features: per k-chunk, fp8 tiles [128, 8, B_CORE] ----
    # slots: 0:t 1:t^2 2:q3 3:g4 4:q5 5:q6 6:q7 7:q8
    SIL = []
    PW = []
    for k in range(N_K):
        xt_t = xload.tile([128, B_CORE], BF16, name=f"xt_{k}", tag="xt")
        nc.sync.dma_start(xt_t, xt_d[128 * k:128 * (k + 1), :])
        sil = feats.tile([128, B_CORE], BF16, name=f"sil_{k}", tag=f"sil{k}")
        nc.scalar.activation(sil, xt_t, AF.Silu)
        pw = feats.tile([128, ORDER, B_CORE], F8, name=f"pw_{k}",
                        tag=f"pw{k}")
        # the chain runs in bf16 (fp8 representation error would be
        # amplified by the cancelling orthogonalization corrections);
        # fp8 copies are cast out only for the matmul feature slots
        tb = scr.tile([128, B_CORE], BF16, name=f"tb_{k}", tag="tb")
        g2b = scr.tile([128, B_CORE], BF16, name=f"g2b_{k}", tag="g2b")
        g4b = scr.tile([128, B_CORE], BF16, name=f"g4b_{k}", tag="g4b")
        u5 = scr.tile([128, B_CORE], BF16, name=f"u5_{k}", tag="u5")
        u6 = scr.tile([128, B_CORE], BF16, name=f"u6_{k}", tag="u6")
        u7 = scr.tile([128, B_CORE], BF16, name=f"u7_{k}", tag="u7")
        u8 = scr.tile([128, B_CORE], BF16, name=f"u8_{k}", tag="u8")
        t_ = pw[:, 0, :]
        g2 = pw[:, 1, :]
        q3 = pw[:, 2, :]
        g4 = pw[:, 3, :]
        q5 = pw[:, 4, :]
        q6 = pw[:, 5, :]
        q7 = pw[:, 6, :]
        q8 = pw[:, 7, :]
        nc.scalar.activation(tb, xt_t, AF.Tanh)
        nc.scalar.activation(g2b, tb, AF.Square)
        nc.scalar.activation(g4b, g2b, AF.Square, bias=bias_b)
        # u8 = 16*(g4+h)^2: the x16 feature scale keeps q8's tiny values
        # out of the fp8 denormal range (coefficients absorb 1/16)
        nc.scalar.activation(u8, g4b, AF.Square, scale=4.0, bias=bias_h)
        nc.scalar.copy(t_, tb)
        nc.vector.tensor_copy(g2, g2b)
        nc.vector.tensor_copy(g4, g4b)
        nc.vector.scalar_tensor_tensor(q3, g2b, a, tb,
                                       op0=ALU.add, op1=ALU.mult)
        nc.gpsimd.tensor_mul(u5, g4b, tb)
        nc.vector.scalar_tensor_tensor(q5, q3, s5, u5,
                                       op0=ALU.mult, op1=ALU.add)
        nc.gpsimd.tensor_mul(u6, g4b, g2b)
        nc.vector.scalar_tensor_tensor(q6, g4, s6, u6,
                                       op0=ALU.mult, op1=ALU.add)
        nc.gpsimd.tensor_mul(u7, g4b, q3)
        nc.vector.scalar_tensor_tensor(q7, q5, s7, u7,
                                       op0=ALU.mult, op1=ALU.add)
        nc.vector.scalar_tensor_tensor(q8, q6, SIG8 * s8, u8,
                                       op0=ALU.mult, op1=ALU.add)
        SIL.append(sil)
        PW.append(pw)
        if dbg is not None and k == 0:
            nc.sync.dma_start(dbg["pw0"], pw)

    z = [zpark.tile([128, OUT_F], F32, name=f"z_{j}", tag=f"z_{j}")
         for j in range(N_J)]

    # ---- matmuls: two groups of 4 row-banks, both o-halves in flight ----
    # (4 j-banks x 2 o-halves = 8 PSUM banks).  Group 0's LayerNorm and
    # output DMA overlap group 1's matmuls, and each lhsT weight-load
    # serves two matmuls (one per o-half).  Row sums / square-sums ride
    # along the park ops via accum_out, so the post-matmul LayerNorm tail
    # is just a few [128,4] DVE ops.
    sm = stat.tile([128, N_J, 2], F32, name="sm", tag="sm")
    s2 = stat.tile([128, N_J, 2], F32, name="s2", tag="s2")
    r = stat.tile([128, N_J], F32, name="r", tag="r")
    nb = stat.tile([128, N_J], F32, name="nb", tag="nb")
    for jg in range(2):
        jlo = 4 * jg
        ps = {(j, oh): psum.tile([128, 512], F32, name=f"ps_{jg}_{j}_{oh}",
                                 tag=f"ps_{j - jlo}_{oh}")
              for j in range(jlo, jlo + 4) for oh in range(N_OH)}
        for k in range(N_K):
            dm_t = dstr.tile([128, N_OH, N_PAIR, 2, 512], F8,
                             name=f"dm_{jg}_{k}", tag="dm")
            nc.sync.dma_start(dm_t, dm_d[k])
            wt_t = dstr.tile([128, N_OH, 512], BF16, name=f"wt_{jg}_{k}",
                             tag="wt")
            nc.sync.dma_start(wt_t, wt_d[k])

            if k == 0:
                # K=1 ones-matmuls inject the bias row; they write every
                # element -> start each bank's accumulation group, and
                # need no feature/DMA inputs, so PE starts immediately.
                for j in range(jlo, jlo + 4):
                    for oh in range(N_OH):
                        nc.tensor.matmul(ps[j, oh], ones_t,
                                         v_t[:, 512 * oh:512 * (oh + 1)],
                                         start=True, stop=False)
            for j in range(jlo, jlo + 4):
                jsl = slice(128 * j, 128 * (j + 1))
                for oh in range(N_OH):
                    nc.tensor.matmul(ps[j, oh], SIL[k][:, jsl], wt_t[:, oh],
                                     start=False, stop=False)
            # pair-major order consumes features in chain-completion order
            for pr in range(N_PAIR):
                for j in range(jlo, jlo + 4):
                    jsl = slice(128 * j, 128 * (j + 1))
                    for oh in range(N_OH):
                        last = (k == N_K - 1 and pr == N_PAIR - 1)
                        nc.tensor.matmul(
                            ps[j, oh], PW[k][:, 2 * pr:2 * pr + 2, jsl],
                            dm_t[:, oh, pr], start=False, stop=last,
                            perf_mode=DR)
        # park + stats spread across ACT/DVE/Pool; per-j LayerNorm chain so
        # each row-tile's final Silu starts as soon as its own stats land
        for j in range(jlo, jlo + 4):
            for oh in range(N_OH):
                p = 2 * (j - jlo) + oh
                osl = slice(512 * oh, 512 * (oh + 1))
                if p % 2 == 0:
                    nc.scalar.activation(z[j][:, osl], ps[j, oh], AF.Copy,
                                         accum_out=sm[:, j, oh:oh + 1])
                else:
                    nc.vector.scalar_tensor_tensor(
                        z[j][:, osl], ps[j, oh], 0.0, zero_t,
                        op0=ALU.add, op1=ALU.add,
                        accum_out=sm[:, j, oh:oh + 1])
                sq = scr.tile([128, 512], F32, name=f"sq_{jg}_{j}_{oh}",
                              tag="sq")
                if p % 2 == 0:
                    # paired with the ACT park above -> square on DVE
                    nc.vector.scalar_tensor_tensor(
                        sq, z[j][:, osl], 1.0, z[j][:, osl],
                        op0=ALU.mult, op1=ALU.mult,
                        accum_out=s2[:, j, oh:oh + 1])
                else:
                    nc.scalar.activation(sq, z[j][:, osl], AF.Square,
                                         accum_out=s2[:, j, oh:oh + 1])

            # per-j LayerNorm stats (scale-invariant; eps scaled)
            js = slice(j, j + 1)
            mean = stat.tile([128, 1], F32, name=f"mean_{j}", tag=f"mean{j}")
            nc.vector.tensor_add(mean, sm[:, j, 0:1], sm[:, j, 1:2])
            nc.vector.tensor_scalar_mul(mean, mean, 1.0 / OUT_F)
            m2 = stat.tile([128, 1], F32, name=f"m2_{j}", tag=f"m2{j}")
            nc.vector.tensor_mul(m2, mean, mean)
            ve = stat.tile([128, 1], F32, name=f"ve_{j}", tag=f"ve{j}")
            nc.vector.tensor_add(ve, s2[:, j, 0:1], s2[:, j, 1:2])
            nc.vector.scalar_tensor_tensor(ve, ve, 1.0 / OUT_F, m2,
                                           op0=ALU.mult, op1=ALU.subtract)
            nc.vector.tensor_scalar_add(ve, ve, LN_EPS * SC * SC)
            # Newton rsqrt: y0 = bitcast(magic - (bits(v) >> 1)), 2 iters
            rj = r[:, js]
            w0 = stat.tile([128, 1], F32, name=f"w0_{j}", tag=f"w0{j}")
            nc.vector.tensor_scalar(w0.bitcast(U32), ve.bitcast(U32), 1,
                                    None, op0=ALU.logical_shift_right)
            nc.vector.tensor_sub(rj.bitcast(U32),
                                 magic_t[:, js].bitcast(U32),
                                 w0.bitcast(U32))
            for _ in range(2):
                nc.vector.tensor_mul(w0, ve, rj)
                nc.vector.tensor_mul(w0, w0, rj)
                nc.vector.tensor_scalar(w0, w0, -0.5, 1.5,
                                        op0=ALU.mult, op1=ALU.add)
                nc.vector.tensor_mul(rj, rj, w0)
            nc.vector.scalar_tensor_tensor(nb[:, js], mean, -1.0, rj,
                                           op0=ALU.mult, op1=ALU.mult)

            o_t = outp.tile([128, OUT_F], F32, name=f"o_{j}", tag="o")
            if general_ln:
                zn = outp.tile([128, OUT_F], F32, name=f"zn_{j}", tag="zn")
                nc.scalar.activation(zn, z[j], AF.Identity,
                                     bias=nb[:, j:j + 1], scale=r[:, j:j + 1])
                nc.vector.tensor_mul(zn, zn, lnw_t)
                nc.vector.tensor_add(zn, zn, lnb_t)
                nc.scalar.activation(o_t, zn, AF.Silu)
            else:
                nc.scalar.activation(o_t, z[j], AF.Silu,
                                     bias=nb[:, j:j + 1], scale=r[:, j:j + 1])
            nc.sync.dma_start(out_d[128 * j:128 * (j + 1), :], o_t)
            if dbg is not None and j == 0:
                nc.sync.dma_start(dbg["z0"], z[0])
    if dbg is not None:
        nc.sync.dma_start(dbg["sm"], sm)
        nc.sync.dma_start(dbg["s2"], s2)
        nc.sync.dma_start(dbg["r"], r)
        nc.sync.dma_start(dbg["nb"], nb)


_PROG_CACHE = {}


def _get_program(general_ln):
    if general_ln not in _PROG_CACHE:
        _PROG_CACHE[general_ln] = _build_program(general_ln)
    return _PROG_CACHE[general_ln]


def _core_xt(x, c):
    xt = x[B_CORE * c:B_CORE * (c + 1), :].T
    return np.ascontiguousarray(xt.astype(ml_dtypes.bfloat16))


def kernel(x, base_weights, jacobi_coeff, ln_weight, ln_bias):
    x = np.asarray(x, np.float32).reshape(B_FULL, IN_F)
    base_weights = np.asarray(base_weights, np.float32)
    jacobi_coeff = np.asarray(jacobi_coeff, np.float32)
    ln_weight = np.asarray(ln_weight, np.float32)
    ln_bias = np.asarray(ln_bias, np.float32)

    general_ln = not (np.all(ln_weight == 1.0) and np.all(ln_bias == 0.0))

    nc = _get_program(general_ln)
    shared = _prep_shared(base_weights, jacobi_coeff, ln_weight, ln_bias,
                          general_ln)

    in_maps = [{"xt": _core_xt(x, c), **shared} for c in range(N_CORES)]

    res = run_bass_kernel_spmd(nc, in_maps, core_ids=list(range(N_CORES)))
    out = np.concatenate([res.results[c]["out"] for c in range(N_CORES)],
                         axis=0)
    return out.astype(np.float32)


if __name__ == "__main__":
    rng = np.random.default_rng(1)
    demo = {
        "x": rng.standard_normal((B_FULL, IN_F)).astype(np.float32),
        "base_weights": rng.standard_normal((OUT_F, IN_F)).astype(np.float32) * 0.04,
        "jacobi_coeff": (rng.standard_normal((IN_F, OUT_F, ORDER + 1))
                         / (IN_F * (ORDER + 1))).astype(np.float32),
        "ln_weight": np.ones(OUT_F, np.float32),
        "ln_bias": np.zeros(OUT_F, np.float32),
    }
    o = kernel(**demo)
    print("kernel output:", o.shape, o.dtype, float(np.abs(o).mean()))
